# revision 10
# baseline (speedup 1.0000x reference)
"""Trainium2 Bass kernel for nn_BlockAttention (block-local attention with RoPE + gate).

Sharding: sequence-parallel over 8 cores. Flattened [B*S=8192, E] rows split into
8 contiguous shards of 1024 rows (4 blocks of 256; blocks never cross cores or
batch boundaries since 4096/256=16 blocks per batch, 4 per core).

Per-core layout strategy (features-on-partitions, "transposed" activations):
  - host pre-transposes the x shard to xT [E, R] so no on-chip transposes needed
  - qT/kT/gateT [E, R] = W.T @ x via matmul(lhsT=W_chunk, rhs=xT_chunk)  (fp32r)
  - v [R, E] natural via matmul(lhsT=xT_chunk, rhs=Wv_chunk)
  - RoPE applied on transposed q/k with host-prepared cos/sin tables
    (replicated per head-pair, rotate-sign folded into sin table)
  - block-local attention per (block, head) with transposed scores S_T[k,q]:
    exp on ScalarE (no max subtraction needed: |scores/8| < ~15), row-sums via
    M=1 ones-matmul on PE, AV via matmul(lhsT=v_block, rhs=expS_T),
    softmax normalize via K=1 ones-outer-product replicate matmul + DVE mul
  - gate: sigmoid on ScalarE, fused multiply on DVE
  - out projection back through Wo in transposed layout; host un-transposes
"""
import sys

sys.path.insert(0, "/opt/trn_rl_repo")
import numpy as np

B, S, E = 2, 4096, 1024
H, D = 16, 64
BLK = 256
NCORES = 8
R = (B * S) // NCORES   # 1024 rows per core
NB = R // BLK           # 4 blocks per core
NCH = E // 128          # 8 feature chunks of 128
SCALE = 1.0 / np.sqrt(D)


def emit(tc, outs, ins):
    """Emit the per-core program. ins/outs are DRAM APs:
    ins  = [xT, wq, wk, wv, wg, wo, cos2, sin2]
    outs = [outT]
    """
    from contextlib import ExitStack
    import concourse.mybir as mybir

    F32 = mybir.dt.float32
    F32R = mybir.dt.float32r
    AF = mybir.ActivationFunctionType

    nc = tc.nc
    xT_d, wq_d, wk_d, wv_d, wg_d, wo_d, c2_d, s2_d = ins
    (outT_d,) = outs

    with ExitStack() as ctx:
        ep = ctx.enter_context
        consts = ep(tc.tile_pool(name="consts", bufs=1))
        big = ep(tc.tile_pool(name="big", bufs=1))
        wpool = ep(tc.tile_pool(name="wpool", bufs=2))
        wvpool = ep(tc.tile_pool(name="wvpool", bufs=1))
        ropet = ep(tc.tile_pool(name="ropet", bufs=2))
        espool = ep(tc.tile_pool(name="espool", bufs=2))
        smalls = ep(tc.tile_pool(name="smalls", bufs=1))
        rrsbp = ep(tc.tile_pool(name="rrsbp", bufs=1))
        opool = ep(tc.tile_pool(name="opool", bufs=2))
        mm_ps = ep(tc.tile_pool(name="mm_ps", bufs=2, space="PSUM"))
        s_ps_p = ep(tc.tile_pool(name="s_ps_p", bufs=2, space="PSUM"))
        av_ps_p = ep(tc.tile_pool(name="av_ps_p", bufs=2, space="PSUM"))
        sum_ps_p = ep(tc.tile_pool(name="sum_ps_p", bufs=1, space="PSUM"))
        rr_ps_p = ep(tc.tile_pool(name="rr_ps_p", bufs=1, space="PSUM"))

        # ---- constants / inputs resident in SBUF
        xt = big.tile([128, NCH, R], F32R, tag="xy")
        for kc in range(NCH):
            nc.sync.dma_start(xt[:, kc, :],
                              xT_d[kc * 128:(kc + 1) * 128, :].bitcast(F32R))
        c2 = consts.tile([128, R], F32)
        nc.sync.dma_start(c2[:], c2_d[:])
        s2 = consts.tile([128, R], F32)
        nc.sync.dma_start(s2[:], s2_d[:])
        onesf = consts.tile([128, 1], F32)
        nc.vector.memset(onesf[:], 1.0)
        ones = consts.tile([128, 1], F32R)
        nc.scalar.activation(ones[:], onesf[:], AF.Copy)
        onesrowf = consts.tile([1, 64], F32)
        nc.vector.memset(onesrowf[:], 1.0)
        onesrow = consts.tile([1, 64], F32R)
        nc.scalar.activation(onesrow[:], onesrowf[:], AF.Copy)

        qT = big.tile([128, NCH, R], F32R)
        kT = big.tile([128, NCH, R], F32R)
        v = big.tile([128, NCH, R], F32R)
        sg = big.tile([128, NCH, R], F32)

        # ---- projections q/k (with fused RoPE) and gate (fused sigmoid)
        def proj_t(w_d, dst, rope):
            for mc in range(NCH):
                w = wpool.tile([128, NCH, 128], F32R, tag="w")
                src = w_d.rearrange("(kc p) m -> p kc m", p=128)
                nc.sync.dma_start(
                    w[:], src[:, :, mc * 128:(mc + 1) * 128].bitcast(F32R))
                for nh in range(2):
                    ps = mm_ps.tile([128, 512], F32, tag="mm")
                    for kc in range(NCH):
                        nc.tensor.matmul(
                            ps[:],
                            w[:, kc, :].bitcast(F32R),
                            xt[:, kc, nh * 512:(nh + 1) * 512].bitcast(F32R),
                            start=(kc == 0),
                            stop=(kc == NCH - 1),
                        )
                    dstsl = dst[:, mc, nh * 512:(nh + 1) * 512]
                    if rope:
                        t = ropet.tile([128, 512], F32, tag="t")
                        for h2 in (0, 64):
                            nc.scalar.activation(
                                t[h2:h2 + 32, :], ps[h2 + 32:h2 + 64, :], AF.Copy)
                            nc.scalar.activation(
                                t[h2 + 32:h2 + 64, :], ps[h2:h2 + 32, :], AF.Copy)
                        nc.vector.tensor_mul(
                            dstsl, ps[:], c2[:, nh * 512:(nh + 1) * 512])
                        nc.vector.tensor_mul(
                            t[:], t[:], s2[:, nh * 512:(nh + 1) * 512])
                        nc.vector.tensor_add(dstsl, dstsl.bitcast(F32), t[:])
                    else:
                        nc.scalar.activation(dstsl, ps[:], AF.Sigmoid)

        proj_t(wq_d, qT, rope=True)
        proj_t(wk_d, kT, rope=True)

        # ---- v projection (natural layout: rows on partitions)
        for nq in range(4):
            wvb = wvpool.tile([128, NCH, 256], F32R, tag="wv")
            for kc in range(NCH):
                nc.sync.dma_start(
                    wvb[:, kc, :],
                    wv_d[kc * 128:(kc + 1) * 128,
                         nq * 256:(nq + 1) * 256].bitcast(F32R))
            for rc in range(NCH):
                ps = mm_ps.tile([128, 512], F32, tag="mm")
                for kc in range(NCH):
                    nc.tensor.matmul(
                        ps[:, 0:256],
                        xt[:, kc, rc * 128:(rc + 1) * 128].bitcast(F32R),
                        wvb[:, kc, :].bitcast(F32R),
                        start=(kc == 0),
                        stop=(kc == NCH - 1),
                    )
                # v free-dim layout: [rc, feat]; feat quarter at offset
                nc.vector.tensor_copy(
                    v[:, rc, nq * 256:(nq + 1) * 256], ps[:, 0:256])

        proj_t(wg_d, sg, rope=False)

        # ---- block-local attention
        # y aliases xt's storage (tag "xy", bufs=1): first write waits for the
        # gate projection's last read of xt.
        y = big.tile([128, NCH, R], F32R, tag="xy")

        for b in range(NB):
            for c in range(NCH):
                ssum = smalls.tile([1, 512], F32, tag="ssum")
                recip = smalls.tile([1, 512], F32R, tag="recip")
                avt = [None, None]
                for hi in range(2):
                    h = 2 * c + hi
                    pb = 64 * hi
                    # scores S_T[k, q] for this block/head (2 kpos halves)
                    sps = s_ps_p.tile([128, 512], F32, tag="s")
                    for kph in range(2):
                        nc.tensor.matmul(
                            sps[:, kph * 256:(kph + 1) * 256],
                            kT[pb:pb + 64, c,
                               b * 256 + kph * 128:b * 256 + (kph + 1) * 128
                               ].bitcast(F32R),
                            qT[pb:pb + 64, c,
                               b * 256:(b + 1) * 256].bitcast(F32R),
                            start=True, stop=True,
                        )
                    es = espool.tile([128, 512], F32R, tag="es")
                    nc.scalar.activation(es[:], sps[:], AF.Exp, scale=float(SCALE))
                    # row sums over k: accumulate both kpos halves on PE
                    sums = sum_ps_p.tile([1, 256], F32, tag="sums")
                    for kph in range(2):
                        nc.tensor.matmul(
                            sums[:],
                            ones[:].bitcast(F32R),
                            es[:, kph * 256:(kph + 1) * 256].bitcast(F32R),
                            start=(kph == 0), stop=(kph == 1),
                        )
                    nc.vector.tensor_copy(
                        ssum[0:1, hi * 256:(hi + 1) * 256], sums[:])
                    # AV: av_T[d, q], accumulate over the 2 kpos chunks
                    av = av_ps_p.tile([64, 256], F32, tag="av")
                    for kph in range(2):
                        nc.tensor.matmul(
                            av[:],
                            v[:, 2 * b + kph, h * 64:(h + 1) * 64].bitcast(F32R),
                            es[:, kph * 256:(kph + 1) * 256].bitcast(F32R),
                            start=(kph == 0), stop=(kph == 1),
                        )
                    avt[hi] = av
                # normalize + write y chunk for this (b, c)
                with nc.allow_low_precision("fp32r softmax denominators"):
                    nc.vector.reciprocal(recip[:], ssum[:])
                rr = rr_ps_p.tile([64, 512], F32, tag="rr")
                for hi in range(2):
                    nc.tensor.matmul(
                        rr[:, hi * 256:(hi + 1) * 256],
                        onesrow[:],
                        recip[0:1, hi * 256:(hi + 1) * 256],
                        start=True, stop=True,
                    )
                rrsb = rrsbp.tile([128, 256], F32, tag="rrsb")
                nc.scalar.activation(rrsb[0:64, :], rr[0:64, 0:256], AF.Copy)
                nc.scalar.activation(rrsb[64:128, :], rr[0:64, 256:512], AF.Copy)
                nc.vector.tensor_mul(
                    y[0:64, c, b * 256:(b + 1) * 256], avt[0][:], rrsb[0:64, :])
                nc.vector.tensor_mul(
                    y[64:128, c, b * 256:(b + 1) * 256], avt[1][:],
                    rrsb[64:128, :])

        # ---- gate multiply
        for c in range(NCH):
            nc.vector.tensor_mul(y[:, c, :], y[:, c, :].bitcast(F32), sg[:, c, :])

        # ---- output projection (transposed): outT[of, r] = Wo.T @ y
        for oc in range(NCH):
            w = wpool.tile([128, NCH, 128], F32R, tag="w")
            src = wo_d.rearrange("(kc p) m -> p kc m", p=128)
            nc.sync.dma_start(
                w[:], src[:, :, oc * 128:(oc + 1) * 128].bitcast(F32R))
            for nh in range(2):
                ps = mm_ps.tile([128, 512], F32, tag="mm")
                for yc in range(NCH):
                    nc.tensor.matmul(
                        ps[:],
                        w[:, yc, :].bitcast(F32R),
                        y[:, yc, nh * 512:(nh + 1) * 512].bitcast(F32R),
                        start=(yc == 0),
                        stop=(yc == NCH - 1),
                    )
                osb = opool.tile([128, 512], F32, tag="o")
                nc.scalar.activation(osb[:], ps[:], AF.Copy)
                nc.sync.dma_start(
                    outT_d[oc * 128:(oc + 1) * 128,
                           nh * 512:(nh + 1) * 512], osb[:])


def _build_nc():
    import concourse.bacc as bacc
    import concourse.mybir as mybir
    import concourse.tile as tile

    F32 = mybir.dt.float32
    nc = bacc.Bacc("TRN2", target_bir_lowering=False, debug=False)
    names_in = ["xT", "wq", "wk", "wv", "wg", "wo", "cos2", "sin2"]
    shapes_in = [[E, R], [E, E], [E, E], [E, E], [E, E], [E, E],
                 [128, R], [128, R]]
    ins = [
        nc.dram_tensor(n, s, F32, kind="ExternalInput").ap()
        for n, s in zip(names_in, shapes_in)
    ]
    outT = nc.dram_tensor("outT", [E, R], F32, kind="ExternalOutput").ap()
    with tile.TileContext(nc) as tc:
        emit(tc, [outT], ins)
    nc.compile()
    return nc


_NC_CACHE = {}


def host_prep(x, Wq, Wk, Wv, Wg, Wo, cos, sin):
    """Build the 8 per-core input maps."""
    x_flat = np.ascontiguousarray(x.reshape(B * S, E), dtype=np.float32)
    Wq = np.ascontiguousarray(Wq, dtype=np.float32)
    Wk = np.ascontiguousarray(Wk, dtype=np.float32)
    Wv = np.ascontiguousarray(Wv, dtype=np.float32)
    Wg = np.ascontiguousarray(Wg, dtype=np.float32)
    Wo = np.ascontiguousarray(Wo, dtype=np.float32)
    cos = np.asarray(cos, dtype=np.float32)
    sin = np.asarray(sin, dtype=np.float32)
    sign = np.where(np.arange(D) < D // 2, -1.0, 1.0).astype(np.float32)

    in_maps = []
    for cix in range(NCORES):
        rows = slice(cix * R, (cix + 1) * R)
        xT = np.ascontiguousarray(x_flat[rows].T)
        seq = (cix * R + np.arange(R)) % S
        cS = cos[seq]            # [R, D]
        sS = sin[seq] * sign     # [R, D] signed
        c2 = np.ascontiguousarray(np.tile(cS.T, (2, 1)))   # [128, R]
        s2 = np.ascontiguousarray(np.tile(sS.T, (2, 1)))   # [128, R]
        in_maps.append({
            "xT": xT, "wq": Wq, "wk": Wk, "wv": Wv, "wg": Wg, "wo": Wo,
            "cos2": c2, "sin2": s2,
        })
    return in_maps


def kernel_traced(x, Wq, Wk, Wv, Wg, Wo, cos, sin, block_size, trace=False,
                  **run_kwargs):
    assert int(block_size) == BLK
    from concourse import bass_utils

    if "nc" not in _NC_CACHE:
        _NC_CACHE["nc"] = _build_nc()
    nc = _NC_CACHE["nc"]

    in_maps = host_prep(x, Wq, Wk, Wv, Wg, Wo, cos, sin)
    res = bass_utils.run_bass_kernel_spmd(
        nc, in_maps, core_ids=list(range(NCORES)), trace=trace, **run_kwargs)
    out_flat = np.empty((B * S, E), dtype=np.float32)
    for cix in range(NCORES):
        out_flat[cix * R:(cix + 1) * R] = res.results[cix]["outT"].T
    return out_flat.reshape(B, S, E), res


def kernel(x, Wq, Wk, Wv, Wg, Wo, cos, sin, block_size):
    return kernel_traced(x, Wq, Wk, Wv, Wg, Wo, cos, sin, block_size)[0]


# revision 11
# speedup vs baseline: 1.1306x; 1.1306x over previous
"""Trainium2 Bass kernel for nn_BlockAttention (block-local attention with RoPE + gate).

Sharding: sequence-parallel over 8 cores. Flattened [B*S=8192, E] rows split into
8 contiguous shards of 1024 rows (4 blocks of 256; blocks never cross cores or
batch boundaries since 4096/256=16 blocks per batch, 4 per core).

Per-core layout strategy (features-on-partitions, "transposed" activations):
  - host pre-transposes the x shard to xT [E, R] so no on-chip transposes needed
  - qT/kT/gateT [E, R] = W.T @ x via matmul(lhsT=W_chunk, rhs=xT_chunk)  (fp32r)
  - v [R, E] natural via matmul(lhsT=xT_chunk, rhs=Wv_chunk)
  - RoPE applied on transposed q/k with host-prepared cos/sin tables
    (replicated per head-pair, rotate-sign folded into sin table)
  - block-local attention per (block, head) with transposed scores S_T[k,q]:
    exp on ScalarE (no max subtraction needed: |scores/8| < ~15), row-sums via
    M=1 ones-matmul on PE, AV via matmul(lhsT=v_block, rhs=expS_T),
    softmax normalize via K=1 ones-outer-product replicate matmul + DVE mul
  - gate: sigmoid on ScalarE, fused multiply on DVE
  - out projection back through Wo in transposed layout; host un-transposes
"""
import sys

sys.path.insert(0, "/opt/trn_rl_repo")
import numpy as np

B, S, E = 2, 4096, 1024
H, D = 16, 64
BLK = 256
NCORES = 8
R = (B * S) // NCORES   # 1024 rows per core
NB = R // BLK           # 4 blocks per core
NCH = E // 128          # 8 feature chunks of 128
SCALE = 1.0 / np.sqrt(D)


def emit(tc, outs, ins):
    """Emit the per-core program. ins/outs are DRAM APs:
    ins  = [xT, wq, wk, wv, wg, wo, cos2, sin2]
    outs = [outT]
    """
    from contextlib import ExitStack
    import concourse.mybir as mybir

    F32 = mybir.dt.float32
    F32R = mybir.dt.float32r
    AF = mybir.ActivationFunctionType

    nc = tc.nc
    xT_d, wq_d, wk_d, wv_d, wg_d, wo_d, c2_d, s2_d = ins
    (outT_d,) = outs

    with ExitStack() as ctx:
        ep = ctx.enter_context
        consts = ep(tc.tile_pool(name="consts", bufs=1))
        big = ep(tc.tile_pool(name="big", bufs=1))
        wpool = ep(tc.tile_pool(name="wpool", bufs=2))
        wvpool = ep(tc.tile_pool(name="wvpool", bufs=1))
        ropet = ep(tc.tile_pool(name="ropet", bufs=2))
        espool = ep(tc.tile_pool(name="espool", bufs=2))
        smalls = ep(tc.tile_pool(name="smalls", bufs=1))
        rrsbp = ep(tc.tile_pool(name="rrsbp", bufs=1))
        opool = ep(tc.tile_pool(name="opool", bufs=2))
        mm_ps = ep(tc.tile_pool(name="mm_ps", bufs=2, space="PSUM"))
        s_ps_p = ep(tc.tile_pool(name="s_ps_p", bufs=2, space="PSUM"))
        av_ps_p = ep(tc.tile_pool(name="av_ps_p", bufs=2, space="PSUM"))
        sum_ps_p = ep(tc.tile_pool(name="sum_ps_p", bufs=1, space="PSUM"))
        rr_ps_p = ep(tc.tile_pool(name="rr_ps_p", bufs=1, space="PSUM"))

        # ---- constants / inputs resident in SBUF
        xt = big.tile([128, NCH, R], F32R, tag="xy")
        for kc in range(NCH):
            nc.sync.dma_start(xt[:, kc, :],
                              xT_d[kc * 128:(kc + 1) * 128, :].bitcast(F32R))
        c2 = consts.tile([128, R], F32)
        nc.sync.dma_start(c2[:], c2_d[:])
        s2 = consts.tile([128, R], F32)
        nc.sync.dma_start(s2[:], s2_d[:])
        onesf = consts.tile([128, 1], F32)
        nc.vector.memset(onesf[:], 1.0)
        ones = consts.tile([128, 1], F32R)
        nc.scalar.activation(ones[:], onesf[:], AF.Copy)
        onesrowf = consts.tile([1, 64], F32)
        nc.vector.memset(onesrowf[:], 1.0)
        onesrow = consts.tile([1, 64], F32R)
        nc.scalar.activation(onesrow[:], onesrowf[:], AF.Copy)

        qT = big.tile([128, NCH, R], F32R)
        kT = big.tile([128, NCH, R], F32R)
        v = big.tile([128, NCH, R], F32R)
        sg = big.tile([128, NCH, R], F32)

        # ---- projections q/k (with fused RoPE) and gate (fused sigmoid)
        def proj_t(w_d, dst, rope):
            for mc in range(NCH):
                w = wpool.tile([128, NCH, 128], F32R, tag="w")
                src = w_d.rearrange("(kc p) m -> p kc m", p=128)
                nc.sync.dma_start(
                    w[:], src[:, :, mc * 128:(mc + 1) * 128].bitcast(F32R))
                for nh in range(2):
                    ps = mm_ps.tile([128, 512], F32, tag="mm")
                    for kc in range(NCH):
                        nc.tensor.matmul(
                            ps[:],
                            w[:, kc, :].bitcast(F32R),
                            xt[:, kc, nh * 512:(nh + 1) * 512].bitcast(F32R),
                            start=(kc == 0),
                            stop=(kc == NCH - 1),
                        )
                    dstsl = dst[:, mc, nh * 512:(nh + 1) * 512]
                    if rope:
                        t = ropet.tile([128, 512], F32, tag="t")
                        for h2 in (0, 64):
                            nc.scalar.activation(
                                t[h2:h2 + 32, :], ps[h2 + 32:h2 + 64, :], AF.Copy)
                            nc.scalar.activation(
                                t[h2 + 32:h2 + 64, :], ps[h2:h2 + 32, :], AF.Copy)
                        nc.vector.tensor_mul(
                            dstsl, ps[:], c2[:, nh * 512:(nh + 1) * 512])
                        nc.vector.tensor_mul(
                            t[:], t[:], s2[:, nh * 512:(nh + 1) * 512])
                        nc.vector.tensor_add(dstsl, dstsl.bitcast(F32), t[:])
                    else:
                        nc.scalar.activation(dstsl, ps[:], AF.Sigmoid)

        proj_t(wq_d, qT, rope=True)
        proj_t(wk_d, kT, rope=True)

        # ---- v projection (natural layout: rows on partitions)
        for nq in range(4):
            wvb = wvpool.tile([128, NCH, 256], F32R, tag="wv")
            for kc in range(NCH):
                nc.sync.dma_start(
                    wvb[:, kc, :],
                    wv_d[kc * 128:(kc + 1) * 128,
                         nq * 256:(nq + 1) * 256].bitcast(F32R))
            for rc in range(NCH):
                ps = mm_ps.tile([128, 512], F32, tag="mm")
                for kc in range(NCH):
                    nc.tensor.matmul(
                        ps[:, 0:256],
                        xt[:, kc, rc * 128:(rc + 1) * 128].bitcast(F32R),
                        wvb[:, kc, :].bitcast(F32R),
                        start=(kc == 0),
                        stop=(kc == NCH - 1),
                    )
                # v free-dim layout: [rc, feat]; feat quarter at offset
                nc.vector.tensor_copy(
                    v[:, rc, nq * 256:(nq + 1) * 256], ps[:, 0:256])

        proj_t(wg_d, sg, rope=False)

        # ---- block-local attention
        # y aliases xt's storage (tag "xy", bufs=1): first write waits for the
        # gate projection's last read of xt.
        y = big.tile([128, NCH, R], F32R, tag="xy")

        for b in range(NB):
            for c in range(NCH):
                recipf = smalls.tile([1, 512], F32, tag="recipf")
                recip = smalls.tile([1, 512], F32R, tag="recip")
                avt = [None, None]
                for hi in range(2):
                    h = 2 * c + hi
                    pb = 64 * hi
                    # scores S_T[k, q] for this block/head (2 kpos halves)
                    sps = s_ps_p.tile([128, 512], F32, tag="s")
                    for kph in range(2):
                        nc.tensor.matmul(
                            sps[:, kph * 256:(kph + 1) * 256],
                            kT[pb:pb + 64, c,
                               b * 256 + kph * 128:b * 256 + (kph + 1) * 128
                               ].bitcast(F32R),
                            qT[pb:pb + 64, c,
                               b * 256:(b + 1) * 256].bitcast(F32R),
                            start=True, stop=True,
                        )
                    es = espool.tile([128, 512], F32R, tag="es")
                    nc.scalar.activation(es[:], sps[:], AF.Exp, scale=float(SCALE))
                    # row sums over k: accumulate both kpos halves on PE
                    sums = sum_ps_p.tile([1, 256], F32, tag="sums")
                    for kph in range(2):
                        nc.tensor.matmul(
                            sums[:],
                            ones[:].bitcast(F32R),
                            es[:, kph * 256:(kph + 1) * 256].bitcast(F32R),
                            start=(kph == 0), stop=(kph == 1),
                        )
                    nc.vector.reciprocal_approx_fast(
                        recipf[0:1, hi * 256:(hi + 1) * 256], sums[:])
                    # AV: av_T[d, q], accumulate over the 2 kpos chunks
                    av = av_ps_p.tile([64, 256], F32, tag="av")
                    for kph in range(2):
                        nc.tensor.matmul(
                            av[:],
                            v[:, 2 * b + kph, h * 64:(h + 1) * 64].bitcast(F32R),
                            es[:, kph * 256:(kph + 1) * 256].bitcast(F32R),
                            start=(kph == 0), stop=(kph == 1),
                        )
                    avt[hi] = av
                # normalize + write y chunk for this (b, c)
                nc.scalar.activation(recip[:], recipf[:], AF.Copy)
                rr = rr_ps_p.tile([64, 512], F32, tag="rr")
                for hi in range(2):
                    nc.tensor.matmul(
                        rr[:, hi * 256:(hi + 1) * 256],
                        onesrow[:],
                        recip[0:1, hi * 256:(hi + 1) * 256],
                        start=True, stop=True,
                    )
                rrsb = rrsbp.tile([128, 256], F32, tag="rrsb")
                nc.scalar.activation(rrsb[0:64, :], rr[0:64, 0:256], AF.Copy)
                nc.scalar.activation(rrsb[64:128, :], rr[0:64, 256:512], AF.Copy)
                nc.vector.tensor_mul(
                    y[0:64, c, b * 256:(b + 1) * 256], avt[0][:], rrsb[0:64, :])
                nc.vector.tensor_mul(
                    y[64:128, c, b * 256:(b + 1) * 256], avt[1][:],
                    rrsb[64:128, :])

        # ---- gate multiply
        for c in range(NCH):
            nc.vector.tensor_mul(y[:, c, :], y[:, c, :].bitcast(F32), sg[:, c, :])

        # ---- output projection (transposed): outT[of, r] = Wo.T @ y
        for oc in range(NCH):
            w = wpool.tile([128, NCH, 128], F32R, tag="w")
            src = wo_d.rearrange("(kc p) m -> p kc m", p=128)
            nc.sync.dma_start(
                w[:], src[:, :, oc * 128:(oc + 1) * 128].bitcast(F32R))
            for nh in range(2):
                ps = mm_ps.tile([128, 512], F32, tag="mm")
                for yc in range(NCH):
                    nc.tensor.matmul(
                        ps[:],
                        w[:, yc, :].bitcast(F32R),
                        y[:, yc, nh * 512:(nh + 1) * 512].bitcast(F32R),
                        start=(yc == 0),
                        stop=(yc == NCH - 1),
                    )
                osb = opool.tile([128, 512], F32, tag="o")
                nc.scalar.activation(osb[:], ps[:], AF.Copy)
                nc.sync.dma_start(
                    outT_d[oc * 128:(oc + 1) * 128,
                           nh * 512:(nh + 1) * 512], osb[:])


def _build_nc():
    import concourse.bacc as bacc
    import concourse.mybir as mybir
    import concourse.tile as tile

    F32 = mybir.dt.float32
    nc = bacc.Bacc("TRN2", target_bir_lowering=False, debug=False)
    names_in = ["xT", "wq", "wk", "wv", "wg", "wo", "cos2", "sin2"]
    shapes_in = [[E, R], [E, E], [E, E], [E, E], [E, E], [E, E],
                 [128, R], [128, R]]
    ins = [
        nc.dram_tensor(n, s, F32, kind="ExternalInput").ap()
        for n, s in zip(names_in, shapes_in)
    ]
    outT = nc.dram_tensor("outT", [E, R], F32, kind="ExternalOutput").ap()
    with tile.TileContext(nc) as tc:
        emit(tc, [outT], ins)
    nc.compile()
    return nc


_NC_CACHE = {}


def host_prep(x, Wq, Wk, Wv, Wg, Wo, cos, sin):
    """Build the 8 per-core input maps."""
    x_flat = np.ascontiguousarray(x.reshape(B * S, E), dtype=np.float32)
    Wq = np.ascontiguousarray(Wq, dtype=np.float32)
    Wk = np.ascontiguousarray(Wk, dtype=np.float32)
    Wv = np.ascontiguousarray(Wv, dtype=np.float32)
    Wg = np.ascontiguousarray(Wg, dtype=np.float32)
    Wo = np.ascontiguousarray(Wo, dtype=np.float32)
    cos = np.asarray(cos, dtype=np.float32)
    sin = np.asarray(sin, dtype=np.float32)
    sign = np.where(np.arange(D) < D // 2, -1.0, 1.0).astype(np.float32)

    in_maps = []
    for cix in range(NCORES):
        rows = slice(cix * R, (cix + 1) * R)
        xT = np.ascontiguousarray(x_flat[rows].T)
        seq = (cix * R + np.arange(R)) % S
        cS = cos[seq]            # [R, D]
        sS = sin[seq] * sign     # [R, D] signed
        c2 = np.ascontiguousarray(np.tile(cS.T, (2, 1)))   # [128, R]
        s2 = np.ascontiguousarray(np.tile(sS.T, (2, 1)))   # [128, R]
        in_maps.append({
            "xT": xT, "wq": Wq, "wk": Wk, "wv": Wv, "wg": Wg, "wo": Wo,
            "cos2": c2, "sin2": s2,
        })
    return in_maps


def kernel_traced(x, Wq, Wk, Wv, Wg, Wo, cos, sin, block_size, trace=False,
                  **run_kwargs):
    assert int(block_size) == BLK
    from concourse import bass_utils

    if "nc" not in _NC_CACHE:
        _NC_CACHE["nc"] = _build_nc()
    nc = _NC_CACHE["nc"]

    in_maps = host_prep(x, Wq, Wk, Wv, Wg, Wo, cos, sin)
    res = bass_utils.run_bass_kernel_spmd(
        nc, in_maps, core_ids=list(range(NCORES)), trace=trace, **run_kwargs)
    out_flat = np.empty((B * S, E), dtype=np.float32)
    for cix in range(NCORES):
        out_flat[cix * R:(cix + 1) * R] = res.results[cix]["outT"].T
    return out_flat.reshape(B, S, E), res


def kernel(x, Wq, Wk, Wv, Wg, Wo, cos, sin, block_size):
    return kernel_traced(x, Wq, Wk, Wv, Wg, Wo, cos, sin, block_size)[0]
